# revision 15
# baseline (speedup 1.0000x reference)
"""Trainium2 Bass kernel for nn_Disc_53515292508892 (ragged_sequence).

Computes: src-GRU (H=1024) over ragged [128,64] token batch -> final hidden,
tgt-GRU seeded with it, then a 2-layer head -> logits [64, 2].
(The reference's ref-encoder outputs are computed then deleted -- dead code --
so they are skipped.)

Sharding: data-parallel over batch, B=64 -> 8 sequences per NeuronCore,
GRU weights replicated, no inter-core communication.

Key optimizations over the 2.35-2.5 ms fp16 baseline (-> 1.62 ms):
  - Whh streams as fp8e4 (x64 scale) moving data against an fp16 hT
    stationary, 4-way PE-column-tiled. Same cycles/col as fp16, but half
    the moving-data power: the HAM power throttle stops demoting the PE
    to 1.2 GHz mid-stream, so the weight streams run at 2.4 GHz
    (measured; keep-warm dummy matmuls made things WORSE -- the throttle
    is power-based, not idle-reset-based. DoubleRow fp8 would halve
    stream cycles but the ISA rejects it with nonzero column tile
    positions, and without column tiling it is a net loss).
    Accuracy: numpy sim rel err 0.0032 (fp8 Whh only), measured 0.0036.
  - Per-gate psum accumulators in stream order r, n, z', with xw and
    biases INJECTED into psum by K=8 identity matmuls at diagonal tile
    positions (32j,32j) / K=1 ones matmuls, all x64-scaled; sigmoid/tanh
    read psum directly with scale=1/64. No pre-activation DVE adds; the
    r-sigmoid and tn2/sn run under the n/z' streams. The r..n stream
    spacing (~870 ns) matches the sig_r handoff+compute latency, so tn2
    is gated equally by both -- reordering streams only hurts.
  - Critical tail transposes the DELTA: hT' = hT + T(e), e = z'*(n-h),
    with the strip-space h' = e + h computed in the DVE slot under the
    PE transposes (fp16 rounding is elementwise => both copies stay
    bit-identical). k-even tiles stream first and the hT accumulate is
    split per chunk so next-step k-even matmuls start on chunk 0.
  - Sequence-length masking folded into the z'-gate bias during phase 1
    (rank-1 matmul injects -20*64 into z' columns past each lane's
    length => z'=sigmoid(-20)~0 => h_new=h); no per-step mask op.
  - Gate columns reordered per group to [r | n | z']; big Whh DMA loads
    deferred past phase-1 issue so they don't stall the xw matmuls.

Layout per core (strips of 4 col-groups x 32 partitions, batch lane b<8):
  h_str [128, 256] f16 : partition 32j+b, free = unit-in-group
  hT8   [128, 2, 128] f16 : transposed state (chunk c, unit p, col 32j+b)
  whh   [128, KT*3072] fp8e4 (x64), column-reordered/sign-folded

Perf ledger (HW exec, NTFF-profiled, this session):
  2.506 ms  prior-session baseline re-measured
  1.909 ms  per-gate psums + injects + mask-fold + scaled psum-read ACT
  1.922 ms  REJECTED 16 keep-warm dummy matmuls (power throttle backlash)
  1.677 ms  fp8 Whh moving data + delta-transpose tail
  1.674 ms  whh DMA deferral, phase-1 psum bufs=3
  1.619 ms  SHIPPED: k-even-first stream order + split hT accumulate
            (re-measured 1.631 ms on a later run; ~1% run-to-run noise)
  1.956 ms  REJECTED resident-wih + 64x fine-grained xT DMAs (per-
            descriptor DMA overhead ~1-2 us starved phase-1)
Step anatomy at 1.62 ms: ~5.7 us/step avg = 2.9 us full-clock stream +
~2.8 us exposed gate chain. The chain is engine-hop-latency bound:
psum-stop->consumer handoffs measure 0.4-0.8 us on HW, and the r/n/z'
stream order already balances the sig_r vs p_n handoff paths into tn2.
Phase 1 ~150 us (PE-cadence bound at full clock). Further gains need
either fewer cross-engine hops (no DVE FMA / tensor-bias ACT on TRN2)
or dripping tgt phase-1 into the src gate windows (~75 us, untested:
risks HAM power-throttle backlash like the dummy experiment).
"""

import sys
import functools

sys.path.insert(0, "/opt/trn_rl_repo")

import numpy as np
import ml_dtypes
import concourse.mybir as mybir
from concourse import bacc, tile
from concourse.bass_utils import run_bass_kernel_spmd

f8 = mybir.dt.float8e4
f16 = mybir.dt.float16
f32 = mybir.dt.float32
AO = mybir.AluOpType
AF = mybir.ActivationFunctionType
DR = mybir.MatmulPerfMode.DoubleRow

V, D, H = 32000, 512, 1024
T = 128          # steps per GRU (T_SRC = T_TGT = 128)
BL = 8           # batch per core
NCORES = 8
NG = 4           # col-tile groups
GW = 768         # gate columns per group: [r 256 | n 256 | z' 256]
KT = H // 128    # 8 k-tiles over hidden
KD = D // 128    # 4 k-tiles over embedding dim
WSCALE = 64.0    # Whh fp8e4 scale
XSCALE = 64.0    # xw / bias / psum scale (= WSCALE; hT stays fp16)
MASKBIAS = -20.0 * XSCALE  # added to z' cols past length => z' ~ 0
N_DUM = 0        # keep-warm dummies hurt: HAM throttle is power-based
K_ORDER = [0, 2, 4, 6, 1, 3, 5, 7]  # even k-tiles first: they read hT chunk 0 only

# z' columns of each group within the 6 x 512 phase-1 chunks:
# group j's z' block = global cols [768j+512, 768j+768)
_ZCOLS = {1: (0, 256), 2: (256, 512), 4: (0, 256), 5: (256, 512)}


# ----------------------------------------------------------------------------
# host-side weight/layout prep
# ----------------------------------------------------------------------------

def _gate_perm():
    """perm[g] = original Whh/Wih row for reordered gate column g; sign[g] =
    -1 for z' columns (z' = 1 - z = sigmoid(-pre_z)).

    Column order per group j: [r (256) | n (256) | z' (256)]."""
    g = np.arange(3072)
    j = g // GW
    u = g % GW
    blk = u // 256          # 0 = r, 1 = n, 2 = z'
    p = u % 256
    row = np.where(blk == 0, 256 * j + p,
                   np.where(blk == 1, 2048 + 256 * j + p,
                            1024 + 256 * j + p))
    sign = np.where(blk == 2, -1.0, 1.0).astype(np.float32)
    is_rz = blk != 1
    return row, sign, is_rz


def _prep_shared(inputs, n_steps):
    """Core-independent tensors (weights, biases, identity)."""
    row, sign, is_rz = _gate_perm()
    out = {}
    for g, wih, whh, bih, bhh in (
        ("src", inputs["src_Wih"], inputs["src_Whh"], inputs["src_bih"], inputs["src_bhh"]),
        ("tgt", inputs["tgt_Wih"], inputs["tgt_Whh"], inputs["tgt_bih"], inputs["tgt_bhh"]),
    ):
        whh_a = (whh[row] * sign[:, None] * WSCALE).T.astype(ml_dtypes.float8_e4m3)
        out[f"whh_{g}"] = np.ascontiguousarray(whh_a.reshape(KT, 128, 3072))
        wih_a = (wih[row] * sign[:, None] * XSCALE).T.astype(np.float16)  # [512, 3072]
        out[f"wih_{g}"] = np.ascontiguousarray(wih_a.reshape(KD, 128, 3072))
        bias_vec = XSCALE * (sign * bih[row] + np.where(is_rz, sign * bhh[row], 0.0))
        out[f"bias_{g}"] = np.broadcast_to(
            bias_vec.astype(np.float16), (128, 3072)).copy()
        # 1024*bhh for the n gate, strip layout (only partition 32j used)
        bhhn = np.zeros((128, 256), np.float16)
        for j in range(NG):
            bhhn[32 * j, :] = (XSCALE * bhh[2048 + 256 * j:2048 + 256 * (j + 1)]).astype(np.float16)
        out[f"bhhn_{g}"] = bhhn
    p1 = inputs["p1_W"].T.reshape(KT, 128, 64).transpose(1, 0, 2).reshape(128, KT * 64)
    out["p1T"] = p1.astype(np.float16)
    out["p1b"] = np.broadcast_to(inputs["p1_b"].astype(np.float16), (128, 64)).copy()
    out["p2T"] = inputs["p2_W"].T.astype(np.float16)              # [64, 2]
    out["p2b"] = np.broadcast_to(inputs["p2_b"].astype(np.float32), (128, 2)).copy()
    out["ident"] = np.eye(128, dtype=np.float16)
    out["ones8"] = np.ones((128, 8), dtype=np.float16)
    out["negrow"] = np.full((1, 256), MASKBIAS, np.float16)
    return out


def _prep_core(inputs, emb16, core, n_steps):
    """Per-core tensors: gathered/transposed token embeddings and length
    mask row-vectors (1.0 where step >= length)."""
    sl = slice(BL * core, BL * (core + 1))
    n_strips = (n_steps * BL + 127) // 128
    out = {}
    for g, ids_key, len_key in (("src", "src", "src_lengths"),
                                ("tgt", "tgt", "tgt_lengths")):
        ids = np.asarray(inputs[ids_key])[:n_steps, sl]           # [T, 8]
        x = emb16[ids]                                            # [T, 8, 512]
        out[f"xT_{g}"] = np.ascontiguousarray(
            x.transpose(2, 0, 1).reshape(KD, 128, n_steps * BL))
        ln = np.asarray(inputs[len_key])[sl]                      # [8]
        q = np.arange(n_strips * 128)
        t = q // BL
        b = q % BL
        mv = (t >= ln[b]).astype(np.float16)                      # [n_strips*128]
        out[f"maskv_{g}"] = mv.reshape(1, n_strips * 128)
    return out


# ----------------------------------------------------------------------------
# device program
# ----------------------------------------------------------------------------

def build_program(n_steps=T, debug=False):
    nc = bacc.Bacc("TRN2", target_bir_lowering=False, debug=False,
                   num_devices=NCORES)
    TB = n_steps * BL
    n_mstrip = (TB + 127) // 128

    dp = nc.declare_dram_parameter
    d_xT = {g: dp(f"xT_{g}", [KD, 128, TB], f16, isOutput=False) for g in ("src", "tgt")}
    d_whh = {g: dp(f"whh_{g}", [KT, 128, 3072], f8, isOutput=False) for g in ("src", "tgt")}
    d_wih = {g: dp(f"wih_{g}", [KD, 128, 3072], f16, isOutput=False) for g in ("src", "tgt")}
    d_bias = {g: dp(f"bias_{g}", [128, 3072], f16, isOutput=False) for g in ("src", "tgt")}
    d_bhhn = {g: dp(f"bhhn_{g}", [128, 256], f16, isOutput=False) for g in ("src", "tgt")}
    d_maskv = {g: dp(f"maskv_{g}", [1, n_mstrip * 128], f16, isOutput=False) for g in ("src", "tgt")}
    d_p1T = dp("p1T", [128, KT * 64], f16, isOutput=False)
    d_p1b = dp("p1b", [128, 64], f16, isOutput=False)
    d_p2T = dp("p2T", [64, 2], f16, isOutput=False)
    d_p2b = dp("p2b", [128, 2], f32, isOutput=False)
    d_ident = dp("ident", [128, 128], f16, isOutput=False)
    d_ones8 = dp("ones8", [128, 8], f16, isOutput=False)
    d_negrow = dp("negrow", [1, 256], f16, isOutput=False)
    d_logits = dp("logits", [BL, 2], f32, isOutput=True)
    if debug:
        d_dbg_h = dp("dbg_h", [128, 256], f16, isOutput=True)
        d_dbg_xw = {g: dp(f"dbg_xw_{g}", [TB, 3072], f16, isOutput=True)
                    for g in ("src", "tgt")}

    with tile.TileContext(nc) as tc:
        with tc.tile_pool(name="const", bufs=1) as cpool, \
             tc.tile_pool(name="work", bufs=2) as wpool, \
             tc.tile_pool(name="xwload", bufs=4) as xwpool, \
             tc.tile_pool(name="p1ev", bufs=4) as evpool, \
             tc.tile_pool(name="psuma", bufs=3, space="PSUM") as psum, \
             tc.tile_pool(name="psumb", bufs=1, space="PSUM") as psumr, \
             tc.tile_pool(name="dram", bufs=1, space="DRAM") as dram:

            # ---- resident constants -------------------------------------
            # phase-1 inputs (xT, masks, bias) load first; the big Whh loads
            # are deferred until after phase-1's instructions so they don't
            # stall the phase-1 matmuls on DMA bandwidth.
            whh8_sb, xT_sb, bias_sb, bhhn_sb, maskv_sb = {}, {}, {}, {}, {}
            for g in ("src", "tgt"):
                xT_sb[g] = cpool.tile([128, KD * TB], f16, tag=f"xT_{g}", name=f"xT_{g}")
                for k in range(KD):
                    nc.sync.dma_start(xT_sb[g][:, TB * k:TB * (k + 1)], d_xT[g][k])
                bias_sb[g] = cpool.tile([128, 3072], f16, tag=f"bias_{g}", name=f"biassb_{g}")
                nc.sync.dma_start(bias_sb[g][:], d_bias[g][:])
                bhhn_sb[g] = cpool.tile([128, 256], f16, tag=f"bhhn_{g}", name=f"bhhnsb_{g}")
                nc.sync.dma_start(bhhn_sb[g][:], d_bhhn[g][:])
                maskv_sb[g] = cpool.tile([1, n_mstrip * 128], f16, tag=f"maskv_{g}",
                                         name=f"maskvsb_{g}")
                nc.sync.dma_start(maskv_sb[g][:], d_maskv[g][:])
                whh8_sb[g] = cpool.tile([128, KT * 3072], f8, tag=f"whh_{g}",
                                        name=f"whh_{g}")
            ident_sb = cpool.tile([128, 128], f16, tag="ident")
            nc.sync.dma_start(ident_sb[:], d_ident[:])
            ones8_sb = cpool.tile([128, 8], f16, tag="ones8")
            nc.sync.dma_start(ones8_sb[:], d_ones8[:])
            negrow_sb = cpool.tile([1, 256], f16, tag="negrow")
            nc.sync.dma_start(negrow_sb[:], d_negrow[:])
            p1T_sb = cpool.tile([128, KT * 64], f16, tag="p1T")
            nc.sync.dma_start(p1T_sb[:], d_p1T[:])
            p1b_sb = cpool.tile([128, 64], f16, tag="p1b")
            nc.sync.dma_start(p1b_sb[:], d_p1b[:])
            p2T_sb = cpool.tile([64, 2], f16, tag="p2T")
            nc.sync.dma_start(p2T_sb[:], d_p2T[:])
            p2b_sb = cpool.tile([128, 2], f32, tag="p2b")
            nc.sync.dma_start(p2b_sb[:], d_p2b[:])

            # ---- phase 1: xw = 1024*(x @ Wih.T + bias), z' cols get the
            # rank-1 length-mask bias; result staged in DRAM ---------------
            xw_dram = {}
            for g in ("src", "tgt"):
                xw_dram[g] = dram.tile([TB, 3072], f16, tag=f"xw_{g}", name=f"xwdram_{g}")

            def p1_units(g):
                for ch in range(6):
                    wihs = []
                    for kd in range(KD):
                        wt = evpool.tile([128, 512], f16, tag="wih_s", name="wih_s",
                                         bufs=8)
                        nc.sync.dma_start(
                            wt[:], d_wih[g][kd, :, 512 * ch:512 * (ch + 1)])
                        wihs.append(wt)
                    zc = _ZCOLS.get(ch)
                    for mi in range(n_mstrip):
                        m0 = 128 * mi
                        msz = min(128, TB - m0)
                        ps = psum.tile([128, 512], f32, tag="p1", name="p1ps")
                        for kd in range(KD):
                            nc.tensor.matmul(
                                ps[0:msz, :],
                                xT_sb[g][:, TB * kd + m0: TB * kd + m0 + msz],
                                wihs[kd][:],
                                start=(kd == 0), stop=(kd == KD - 1 and zc is None),
                            )
                        if zc is not None:
                            nc.tensor.matmul(
                                ps[0:msz, zc[0]:zc[1]],
                                maskv_sb[g][0:1, m0:m0 + msz],
                                negrow_sb[0:1, :],
                                start=False, stop=True,
                            )
                        ev = evpool.tile([128, 512], f16, tag="ev", name="ev")
                        nc.vector.tensor_add(
                            ev[0:msz, :], ps[0:msz, :],
                            bias_sb[g][0:msz, 512 * ch:512 * (ch + 1)])
                        nc.sync.dma_start(
                            xw_dram[g][m0:m0 + msz, 512 * ch:512 * (ch + 1)],
                            ev[0:msz, :])
                        yield

            # src xw is needed at step 0: run serially. tgt xw is not needed
            # until step n_steps: DRIP its units into the src steps' PE idle
            # windows (one unit every other step; 48 units done by step ~96).
            for _ in p1_units("src"):
                pass
            tgt_gen = p1_units("tgt")

            # whh loads overlap the tail of phase-1
            for g in ("src", "tgt"):
                for k in range(KT):
                    nc.sync.dma_start(whh8_sb[g][:, 3072 * k:3072 * (k + 1)], d_whh[g][k])

            # ---- recurrence ---------------------------------------------
            h_str = wpool.tile([128, 256], f16, tag="h_str")
            hT8 = wpool.tile([128, 2, 128], f16, tag="hT8")
            nc.vector.memset(h_str[:], 0.0)
            nc.vector.memset(hT8[:, :, :], 0.0)

            inv_x = 1.0 / XSCALE

            for step in range(2 * n_steps):
                g = "src" if step < n_steps else "tgt"
                t = step % n_steps

                xw_t = xwpool.tile([128, GW], f16, tag="xw_t")
                for j in range(NG):
                    nc.sync.dma_start(
                        xw_t[32 * j:32 * j + BL, :],
                        xw_dram[g][t * BL:(t + 1) * BL, GW * j:GW * (j + 1)])

                p_r = psumr.tile([128, 256], f32, tag="p_r", name="p_r")
                p_n = psumr.tile([128, 256], f32, tag="p_n", name="p_n")
                p_z = psumr.tile([128, 256], f32, tag="p_z", name="p_z")

                # r-gate: xw inject + 4 DoubleRow matmuls
                for j in range(NG):
                    nc.tensor.matmul(
                        p_r[32 * j:32 * j + BL, :],
                        ident_sb[32 * j:32 * j + BL, 32 * j:32 * j + BL],
                        xw_t[32 * j:32 * j + BL, 0:256],
                        start=True, stop=False, tile_position=(32 * j, 32 * j))
                for ki, k in enumerate(K_ORDER):
                    lhsT = hT8[:, k % 2, 32 * (k // 2):32 * (k // 2) + BL]
                    for j in range(NG):
                        nc.tensor.matmul(
                            p_r[32 * j:32 * j + BL, :],
                            lhsT,
                            whh8_sb[g][:, 3072 * k + GW * j:3072 * k + GW * j + 256],
                            start=False, stop=(ki == KT - 1),
                            tile_position=(0, 32 * j))
                # n-gate: bhh_n inject + 4 DoubleRow matmuls
                for j in range(NG):
                    nc.tensor.matmul(
                        p_n[32 * j:32 * j + BL, :],
                        ones8_sb[32 * j:32 * j + 1, 0:BL],
                        bhhn_sb[g][32 * j:32 * j + 1, :],
                        start=True, stop=False, tile_position=(32 * j, 32 * j))
                for ki, k in enumerate(K_ORDER):
                    lhsT = hT8[:, k % 2, 32 * (k // 2):32 * (k // 2) + BL]
                    for j in range(NG):
                        nc.tensor.matmul(
                            p_n[32 * j:32 * j + BL, :],
                            lhsT,
                            whh8_sb[g][:, 3072 * k + GW * j + 256:3072 * k + GW * j + 512],
                            start=False, stop=(ki == KT - 1),
                            tile_position=(0, 32 * j))
                # z'-gate: xw inject (mask-folded) + 4 DoubleRow matmuls
                for j in range(NG):
                    nc.tensor.matmul(
                        p_z[32 * j:32 * j + BL, :],
                        ident_sb[32 * j:32 * j + BL, 32 * j:32 * j + BL],
                        xw_t[32 * j:32 * j + BL, 512:768],
                        start=True, stop=False, tile_position=(32 * j, 32 * j))
                for ki, k in enumerate(K_ORDER):
                    lhsT = hT8[:, k % 2, 32 * (k // 2):32 * (k // 2) + BL]
                    for j in range(NG):
                        nc.tensor.matmul(
                            p_z[32 * j:32 * j + BL, :],
                            lhsT,
                            whh8_sb[g][:, 3072 * k + GW * j + 512:3072 * k + GW * (j + 1)],
                            start=False, stop=(ki == KT - 1),
                            tile_position=(0, 32 * j))

                # drip one tgt phase-1 unit into the gate window of every
                # other src step (real work in the slot where keep-warm
                # dummies caused power-throttle backlash)
                if g == "src" and step % 2 == 0:
                    next(tgt_gen, None)

                # gates (strip view [128, 256]; only partitions 32j+b<8 valid)
                r_t = wpool.tile([128, 256], f16, tag="r_t")
                nc.scalar.activation(r_t[:], p_r[:], AF.Sigmoid, scale=inv_x)
                z_t = wpool.tile([128, 256], f16, tag="z_t")
                nc.scalar.activation(z_t[:], p_z[:], AF.Sigmoid, scale=inv_x)

                # n = tanh((xn + r * (hn + bhn)) / 1024)
                tn2 = wpool.tile([128, 256], f16, tag="tn2")
                nc.vector.tensor_mul(tn2[:], p_n[:], r_t[:])
                sn = wpool.tile([128, 256], f16, tag="sn")
                nc.vector.tensor_add(sn[:], tn2[:], xw_t[:, 256:512])
                n_t = wpool.tile([128, 256], f16, tag="n_t")
                nc.scalar.activation(n_t[:], sn[:], AF.Tanh, scale=inv_x)

                # h' = h + z'*(n - h)   (mask folded into z')
                d_t = wpool.tile([128, 256], f16, tag="d_t")
                nc.vector.tensor_sub(d_t[:], n_t[:], h_str[:])
                e_t = wpool.tile([128, 256], f16, tag="e_t")
                nc.vector.tensor_mul(e_t[:], d_t[:], z_t[:])

                # critical path: transpose the DELTA and accumulate into hT
                # (hT' = hT + e^T); the strip-space update h' = e + h runs in
                # the DVE idle slot while the PE transposes. fp16 rounding of
                # (h + e) is elementwise, so both copies stay bit-identical.
                tp = psumr.tile([128, 2, 128], f16, tag="tp")
                for c in range(2):
                    nc.tensor.transpose(tp[:, c, :],
                                        e_t[:, 128 * c:128 * (c + 1)],
                                        ident_sb[:])
                h_new = wpool.tile([128, 256], f16, tag="h_str", name="h_new")
                nc.vector.tensor_add(h_new[:], e_t[:], h_str[:])
                hT8_new = wpool.tile([128, 2, 128], f16, tag="hT8", name="hT8_new")
                for c in range(2):
                    nc.vector.tensor_add(hT8_new[:, c, :], tp[:, c, :], hT8[:, c, :])

                h_str, hT8 = h_new, hT8_new

            # ---- head ----------------------------------------------------
            ph = psum.tile([128, 512], f32, tag="p1", name="ph")
            for k in range(KT):
                nc.tensor.matmul(
                    ph[0:BL, 0:64],
                    hT8[:, k % 2, 32 * (k // 2):32 * (k // 2) + BL],
                    p1T_sb[:, 64 * k:64 * (k + 1)],
                    start=(k == 0), stop=(k == KT - 1),
                )
            t1s = wpool.tile([128, 64], f16, tag="t1s")
            nc.vector.tensor_add(t1s[0:BL, :], ph[0:BL, 0:64], p1b_sb[0:BL, :])
            t1 = wpool.tile([128, 64], f16, tag="t1")
            nc.scalar.activation(t1[0:BL, :], t1s[0:BL, :], AF.Tanh)

            pt1 = psumr.tile([128, 256], f16, tag="tp", name="pt1")
            nc.tensor.transpose(pt1[0:64, 0:BL], t1[0:BL, 0:64], ident_sb[0:BL, 0:BL])
            t1T = wpool.tile([64, BL], f16, tag="t1T")
            nc.vector.tensor_copy(t1T[:], pt1[0:64, 0:BL])

            pl = psum.tile([128, 512], f32, tag="p1", name="pl")
            nc.tensor.matmul(pl[0:BL, 0:2], t1T[:], p2T_sb[:], start=True, stop=True)
            lg = wpool.tile([128, 2], f32, tag="lg")
            nc.vector.tensor_add(lg[0:BL, :], pl[0:BL, 0:2], p2b_sb[0:BL, :])
            nc.sync.dma_start(d_logits[:], lg[0:BL, :])

            if debug:
                nc.sync.dma_start(d_dbg_h[:], h_str[:])
                for g in ("src", "tgt"):
                    dbg = evpool.tile([128, 3072], f16, tag="dbgxw")
                    for mi in range(n_mstrip):
                        m0 = 128 * mi
                        msz = min(128, TB - m0)
                        nc.sync.dma_start(dbg[0:msz, :], xw_dram[g][m0:m0 + msz, :])
                        nc.sync.dma_start(d_dbg_xw[g][m0:m0 + msz, :], dbg[0:msz, :])

    nc.compile()
    return nc


# ----------------------------------------------------------------------------
# entry point
# ----------------------------------------------------------------------------

@functools.lru_cache(maxsize=2)
def _cached_program(n_steps, debug):
    return build_program(n_steps, debug)


def run(inputs, n_steps=T, debug=False, trace=False):
    inputs = {k: np.asarray(v) for k, v in inputs.items()}
    nc = _cached_program(n_steps, debug)
    shared = _prep_shared(inputs, n_steps)
    emb16 = np.asarray(inputs["emb"]).astype(np.float16)
    in_maps = []
    for c in range(NCORES):
        m = dict(shared)
        m.update(_prep_core(inputs, emb16, c, n_steps))
        in_maps.append(m)
    res = run_bass_kernel_spmd(nc, in_maps, list(range(NCORES)), trace=trace)
    logits = np.concatenate([res.results[c]["logits"] for c in range(NCORES)], axis=0)
    return logits, res


def kernel(**inputs) -> np.ndarray:
    logits, _ = run(inputs)
    return logits.astype(np.float32)


# revision 16
# speedup vs baseline: 1.1589x; 1.1589x over previous
"""Trainium2 Bass kernel for nn_Disc_53515292508892 (ragged_sequence).

Computes: src-GRU (H=1024) over ragged [128,64] token batch -> final hidden,
tgt-GRU seeded with it, then a 2-layer head -> logits [64, 2].
(The reference's ref-encoder outputs are computed then deleted -- dead code --
so they are skipped.)

Sharding: data-parallel over batch, B=64 -> 8 sequences per NeuronCore,
GRU weights replicated, no inter-core communication.

Key optimizations over the 2.35-2.5 ms fp16 baseline (-> 1.62 ms):
  - Whh streams as fp8e4 (x64 scale) moving data against an fp16 hT
    stationary, 4-way PE-column-tiled. Same cycles/col as fp16, but half
    the moving-data power: the HAM power throttle stops demoting the PE
    to 1.2 GHz mid-stream, so the weight streams run at 2.4 GHz
    (measured; keep-warm dummy matmuls made things WORSE -- the throttle
    is power-based, not idle-reset-based. DoubleRow fp8 would halve
    stream cycles but the ISA rejects it with nonzero column tile
    positions, and without column tiling it is a net loss).
    Accuracy: numpy sim rel err 0.0032 (fp8 Whh only), measured 0.0036.
  - Per-gate psum accumulators in stream order r, n, z', with xw and
    biases INJECTED into psum by K=8 identity matmuls at diagonal tile
    positions (32j,32j) / K=1 ones matmuls, all x64-scaled; sigmoid/tanh
    read psum directly with scale=1/64. No pre-activation DVE adds; the
    r-sigmoid and tn2/sn run under the n/z' streams. The r..n stream
    spacing (~870 ns) matches the sig_r handoff+compute latency, so tn2
    is gated equally by both -- reordering streams only hurts.
  - Critical tail transposes the DELTA: hT' = hT + T(e), e = z'*(n-h),
    with the strip-space h' = e + h computed in the DVE slot under the
    PE transposes (fp16 rounding is elementwise => both copies stay
    bit-identical). k-even tiles stream first and the hT accumulate is
    split per chunk so next-step k-even matmuls start on chunk 0.
  - Sequence-length masking folded into the z'-gate bias during phase 1
    (rank-1 matmul injects -20*64 into z' columns past each lane's
    length => z'=sigmoid(-20)~0 => h_new=h); no per-step mask op.
  - Gate columns reordered per group to [r | n | z']; big Whh DMA loads
    deferred past phase-1 issue so they don't stall the xw matmuls.

Layout per core (strips of 4 col-groups x 32 partitions, batch lane b<8):
  h_str [128, 256] f16 : partition 32j+b, free = unit-in-group
  hT8   [128, 2, 128] f16 : transposed state (chunk c, unit p, col 32j+b)
  whh   [128, KT*3072] fp8e4 (x64), column-reordered/sign-folded

Perf ledger (HW exec, NTFF-profiled, this session):
  2.506 ms  prior-session baseline re-measured
  1.909 ms  per-gate psums + injects + mask-fold + scaled psum-read ACT
  1.922 ms  REJECTED 16 keep-warm dummy matmuls (power throttle backlash)
  1.677 ms  fp8 Whh moving data + delta-transpose tail
  1.674 ms  whh DMA deferral, phase-1 psum bufs=3
  1.619 ms  SHIPPED: k-even-first stream order + split hT accumulate
            (re-measured 1.631 ms on a later run; ~1% run-to-run noise)
  1.956 ms  REJECTED resident-wih + 64x fine-grained xT DMAs (per-
            descriptor DMA overhead ~1-2 us starved phase-1)
Step anatomy at 1.62 ms: ~5.7 us/step avg = 2.9 us full-clock stream +
~2.8 us exposed gate chain. The chain is engine-hop-latency bound:
psum-stop->consumer handoffs measure 0.4-0.8 us on HW, and the r/n/z'
stream order already balances the sig_r vs p_n handoff paths into tn2.
Phase 1 ~150 us (PE-cadence bound at full clock). Further gains need
either fewer cross-engine hops (no DVE FMA / tensor-bias ACT on TRN2)
or dripping tgt phase-1 into the src gate windows (~75 us, untested:
risks HAM power-throttle backlash like the dummy experiment).
"""

import sys
import functools

sys.path.insert(0, "/opt/trn_rl_repo")

import numpy as np
import ml_dtypes
import concourse.mybir as mybir
from concourse import bacc, tile
from concourse.bass_utils import run_bass_kernel_spmd

f8 = mybir.dt.float8e4
f16 = mybir.dt.float16
f32 = mybir.dt.float32
AO = mybir.AluOpType
AF = mybir.ActivationFunctionType
DR = mybir.MatmulPerfMode.DoubleRow

V, D, H = 32000, 512, 1024
T = 128          # steps per GRU (T_SRC = T_TGT = 128)
BL = 8           # batch per core
NCORES = 8
NG = 4           # col-tile groups
GW = 768         # gate columns per group: [r 256 | n 256 | z' 256]
KT = H // 128    # 8 k-tiles over hidden
KD = D // 128    # 4 k-tiles over embedding dim
WSCALE = 64.0    # Whh fp8e4 scale
XSCALE = 64.0    # xw / bias / psum scale (= WSCALE; hT stays fp16)
MASKBIAS = -20.0 * XSCALE  # added to z' cols past length => z' ~ 0
N_DUM = 0        # keep-warm dummies hurt: HAM throttle is power-based
K_ORDER = [0, 2, 4, 6, 1, 3, 5, 7]  # even k-tiles first: they read hT chunk 0 only

# z' columns of each group within the 6 x 512 phase-1 chunks:
# group j's z' block = global cols [768j+512, 768j+768)
_ZCOLS = {1: (0, 256), 2: (256, 512), 4: (0, 256), 5: (256, 512)}


# ----------------------------------------------------------------------------
# host-side weight/layout prep
# ----------------------------------------------------------------------------

def _gate_perm():
    """perm[g] = original Whh/Wih row for reordered gate column g; sign[g] =
    -1 for z' columns (z' = 1 - z = sigmoid(-pre_z)).

    Column order per group j: [r (256) | n (256) | z' (256)]."""
    g = np.arange(3072)
    j = g // GW
    u = g % GW
    blk = u // 256          # 0 = r, 1 = n, 2 = z'
    p = u % 256
    row = np.where(blk == 0, 256 * j + p,
                   np.where(blk == 1, 2048 + 256 * j + p,
                            1024 + 256 * j + p))
    sign = np.where(blk == 2, -1.0, 1.0).astype(np.float32)
    is_rz = blk != 1
    return row, sign, is_rz


def _prep_shared(inputs, n_steps):
    """Core-independent tensors (weights, biases, identity)."""
    row, sign, is_rz = _gate_perm()
    out = {}
    for g, wih, whh, bih, bhh in (
        ("src", inputs["src_Wih"], inputs["src_Whh"], inputs["src_bih"], inputs["src_bhh"]),
        ("tgt", inputs["tgt_Wih"], inputs["tgt_Whh"], inputs["tgt_bih"], inputs["tgt_bhh"]),
    ):
        whh_a = (whh[row] * sign[:, None] * WSCALE).T.astype(ml_dtypes.float8_e4m3)
        out[f"whh_{g}"] = np.ascontiguousarray(whh_a.reshape(KT, 128, 3072))
        wih_a = (wih[row] * sign[:, None] * XSCALE).T.astype(np.float16)  # [512, 3072]
        out[f"wih_{g}"] = np.ascontiguousarray(wih_a.reshape(KD, 128, 3072))
        bias_vec = XSCALE * (sign * bih[row] + np.where(is_rz, sign * bhh[row], 0.0))
        out[f"bias_{g}"] = np.broadcast_to(
            bias_vec.astype(np.float16), (128, 3072)).copy()
        # 1024*bhh for the n gate, strip layout (only partition 32j used)
        bhhn = np.zeros((128, 256), np.float16)
        for j in range(NG):
            bhhn[32 * j, :] = (XSCALE * bhh[2048 + 256 * j:2048 + 256 * (j + 1)]).astype(np.float16)
        out[f"bhhn_{g}"] = bhhn
    p1 = inputs["p1_W"].T.reshape(KT, 128, 64).transpose(1, 0, 2).reshape(128, KT * 64)
    out["p1T"] = p1.astype(np.float16)
    out["p1b"] = np.broadcast_to(inputs["p1_b"].astype(np.float16), (128, 64)).copy()
    out["p2T"] = inputs["p2_W"].T.astype(np.float16)              # [64, 2]
    out["p2b"] = np.broadcast_to(inputs["p2_b"].astype(np.float32), (128, 2)).copy()
    out["ident"] = np.eye(128, dtype=np.float16)
    out["ones8"] = np.ones((128, 8), dtype=np.float16)
    out["negrow"] = np.full((1, 256), MASKBIAS, np.float16)
    return out


def _prep_core(inputs, emb16, core, n_steps):
    """Per-core tensors: gathered/transposed token embeddings and length
    mask row-vectors (1.0 where step >= length)."""
    sl = slice(BL * core, BL * (core + 1))
    n_strips = (n_steps * BL + 127) // 128
    out = {}
    for g, ids_key, len_key in (("src", "src", "src_lengths"),
                                ("tgt", "tgt", "tgt_lengths")):
        ids = np.asarray(inputs[ids_key])[:n_steps, sl]           # [T, 8]
        x = emb16[ids]                                            # [T, 8, 512]
        out[f"xT_{g}"] = np.ascontiguousarray(
            x.transpose(2, 0, 1).reshape(KD, 128, n_steps * BL))
        ln = np.asarray(inputs[len_key])[sl]                      # [8]
        q = np.arange(n_strips * 128)
        t = q // BL
        b = q % BL
        mv = (t >= ln[b]).astype(np.float16)                      # [n_strips*128]
        out[f"maskv_{g}"] = mv.reshape(1, n_strips * 128)
    return out


# ----------------------------------------------------------------------------
# device program
# ----------------------------------------------------------------------------

def build_program(n_steps=T, debug=False):
    nc = bacc.Bacc("TRN2", target_bir_lowering=False, debug=False,
                   num_devices=NCORES)
    TB = n_steps * BL
    n_mstrip = (TB + 127) // 128

    dp = nc.declare_dram_parameter
    d_xT = {g: dp(f"xT_{g}", [KD, 128, TB], f16, isOutput=False) for g in ("src", "tgt")}
    d_whh = {g: dp(f"whh_{g}", [KT, 128, 3072], f8, isOutput=False) for g in ("src", "tgt")}
    d_wih = {g: dp(f"wih_{g}", [KD, 128, 3072], f16, isOutput=False) for g in ("src", "tgt")}
    d_bias = {g: dp(f"bias_{g}", [128, 3072], f16, isOutput=False) for g in ("src", "tgt")}
    d_bhhn = {g: dp(f"bhhn_{g}", [128, 256], f16, isOutput=False) for g in ("src", "tgt")}
    d_maskv = {g: dp(f"maskv_{g}", [1, n_mstrip * 128], f16, isOutput=False) for g in ("src", "tgt")}
    d_p1T = dp("p1T", [128, KT * 64], f16, isOutput=False)
    d_p1b = dp("p1b", [128, 64], f16, isOutput=False)
    d_p2T = dp("p2T", [64, 2], f16, isOutput=False)
    d_p2b = dp("p2b", [128, 2], f32, isOutput=False)
    d_ident = dp("ident", [128, 128], f16, isOutput=False)
    d_ones8 = dp("ones8", [128, 8], f16, isOutput=False)
    d_negrow = dp("negrow", [1, 256], f16, isOutput=False)
    d_logits = dp("logits", [BL, 2], f32, isOutput=True)
    if debug:
        d_dbg_h = dp("dbg_h", [128, 256], f16, isOutput=True)
        d_dbg_xw = {g: dp(f"dbg_xw_{g}", [TB, 3072], f16, isOutput=True)
                    for g in ("src", "tgt")}

    with tile.TileContext(nc) as tc:
        with tc.tile_pool(name="const", bufs=1) as cpool, \
             tc.tile_pool(name="work", bufs=2) as wpool, \
             tc.tile_pool(name="xwload", bufs=4) as xwpool, \
             tc.tile_pool(name="p1ev", bufs=4) as evpool, \
             tc.tile_pool(name="psuma", bufs=3, space="PSUM") as psum, \
             tc.tile_pool(name="psumb", bufs=1, space="PSUM") as psumr, \
             tc.tile_pool(name="dram", bufs=1, space="DRAM") as dram:

            # ---- resident constants -------------------------------------
            # phase-1 inputs (xT, masks, bias) load first; the big Whh loads
            # are deferred until after phase-1's instructions so they don't
            # stall the phase-1 matmuls on DMA bandwidth.
            whh8_sb, xT_sb, bias_sb, bhhn_sb, maskv_sb = {}, {}, {}, {}, {}
            for g in ("src", "tgt"):
                xT_sb[g] = cpool.tile([128, KD * TB], f16, tag=f"xT_{g}", name=f"xT_{g}")
                for k in range(KD):
                    nc.sync.dma_start(xT_sb[g][:, TB * k:TB * (k + 1)], d_xT[g][k])
                bias_sb[g] = cpool.tile([128, 3072], f16, tag=f"bias_{g}", name=f"biassb_{g}")
                nc.sync.dma_start(bias_sb[g][:], d_bias[g][:])
                bhhn_sb[g] = cpool.tile([128, 256], f16, tag=f"bhhn_{g}", name=f"bhhnsb_{g}")
                nc.sync.dma_start(bhhn_sb[g][:], d_bhhn[g][:])
                maskv_sb[g] = cpool.tile([1, n_mstrip * 128], f16, tag=f"maskv_{g}",
                                         name=f"maskvsb_{g}")
                nc.sync.dma_start(maskv_sb[g][:], d_maskv[g][:])
                whh8_sb[g] = cpool.tile([128, KT * 3072], f8, tag=f"whh_{g}",
                                        name=f"whh_{g}")
            ident_sb = cpool.tile([128, 128], f16, tag="ident")
            nc.sync.dma_start(ident_sb[:], d_ident[:])
            ones8_sb = cpool.tile([128, 8], f16, tag="ones8")
            nc.sync.dma_start(ones8_sb[:], d_ones8[:])
            negrow_sb = cpool.tile([1, 256], f16, tag="negrow")
            nc.sync.dma_start(negrow_sb[:], d_negrow[:])
            p1T_sb = cpool.tile([128, KT * 64], f16, tag="p1T")
            nc.sync.dma_start(p1T_sb[:], d_p1T[:])
            p1b_sb = cpool.tile([128, 64], f16, tag="p1b")
            nc.sync.dma_start(p1b_sb[:], d_p1b[:])
            p2T_sb = cpool.tile([64, 2], f16, tag="p2T")
            nc.sync.dma_start(p2T_sb[:], d_p2T[:])
            p2b_sb = cpool.tile([128, 2], f32, tag="p2b")
            nc.sync.dma_start(p2b_sb[:], d_p2b[:])

            # ---- phase 1: xw = 1024*(x @ Wih.T + bias), z' cols get the
            # rank-1 length-mask bias; result staged in DRAM ---------------
            xw_dram = {}
            for g in ("src", "tgt"):
                xw_dram[g] = dram.tile([TB, 3072], f16, tag=f"xw_{g}", name=f"xwdram_{g}")

            def p1_units(g):
                for ch in range(6):
                    wihs = []
                    for kd in range(KD):
                        wt = evpool.tile([128, 512], f16, tag="wih_s", name="wih_s",
                                         bufs=8)
                        nc.sync.dma_start(
                            wt[:], d_wih[g][kd, :, 512 * ch:512 * (ch + 1)])
                        wihs.append(wt)
                    zc = _ZCOLS.get(ch)
                    for mi in range(n_mstrip):
                        m0 = 128 * mi
                        msz = min(128, TB - m0)
                        ps = psum.tile([128, 512], f32, tag="p1", name="p1ps")
                        for kd in range(KD):
                            nc.tensor.matmul(
                                ps[0:msz, :],
                                xT_sb[g][:, TB * kd + m0: TB * kd + m0 + msz],
                                wihs[kd][:],
                                start=(kd == 0), stop=(kd == KD - 1 and zc is None),
                            )
                        if zc is not None:
                            nc.tensor.matmul(
                                ps[0:msz, zc[0]:zc[1]],
                                maskv_sb[g][0:1, m0:m0 + msz],
                                negrow_sb[0:1, :],
                                start=False, stop=True,
                            )
                        ev = evpool.tile([128, 512], f16, tag="ev", name="ev")
                        nc.vector.tensor_add(
                            ev[0:msz, :], ps[0:msz, :],
                            bias_sb[g][0:msz, 512 * ch:512 * (ch + 1)])
                        nc.sync.dma_start(
                            xw_dram[g][m0:m0 + msz, 512 * ch:512 * (ch + 1)],
                            ev[0:msz, :])
                        yield

            for g_ in ("src", "tgt"):
                for _ in p1_units(g_):
                    pass

            # whh loads overlap the tail of phase-1
            for g in ("src", "tgt"):
                for k in range(KT):
                    nc.sync.dma_start(whh8_sb[g][:, 3072 * k:3072 * (k + 1)], d_whh[g][k])

            # ---- recurrence ---------------------------------------------
            h_str = wpool.tile([128, 256], f16, tag="h_str")
            hT8 = wpool.tile([128, 2, 128], f16, tag="hT8")
            nc.vector.memset(h_str[:], 0.0)
            nc.vector.memset(hT8[:, :, :], 0.0)

            inv_x = 1.0 / XSCALE

            for step in range(2 * n_steps):
                g = "src" if step < n_steps else "tgt"
                t = step % n_steps

                xw_t = xwpool.tile([128, GW], f16, tag="xw_t")
                for j in range(NG):
                    nc.sync.dma_start(
                        xw_t[32 * j:32 * j + BL, :],
                        xw_dram[g][t * BL:(t + 1) * BL, GW * j:GW * (j + 1)])

                p_r = psumr.tile([128, 256], f32, tag="p_r", name="p_r")
                p_n = psumr.tile([128, 256], f32, tag="p_n", name="p_n")
                p_z = psumr.tile([128, 256], f32, tag="p_z", name="p_z")

                # r-gate: xw inject + 4 DoubleRow matmuls
                for j in range(NG):
                    nc.tensor.matmul(
                        p_r[32 * j:32 * j + BL, :],
                        ident_sb[32 * j:32 * j + BL, 32 * j:32 * j + BL],
                        xw_t[32 * j:32 * j + BL, 0:256],
                        start=True, stop=False, tile_position=(32 * j, 32 * j))
                for ki, k in enumerate(K_ORDER):
                    lhsT = hT8[:, k % 2, 32 * (k // 2):32 * (k // 2) + BL]
                    for j in range(NG):
                        nc.tensor.matmul(
                            p_r[32 * j:32 * j + BL, :],
                            lhsT,
                            whh8_sb[g][:, 3072 * k + GW * j:3072 * k + GW * j + 256],
                            start=False, stop=(ki == KT - 1),
                            tile_position=(0, 32 * j))
                # n-gate: bhh_n inject + 4 DoubleRow matmuls
                for j in range(NG):
                    nc.tensor.matmul(
                        p_n[32 * j:32 * j + BL, :],
                        ones8_sb[32 * j:32 * j + 1, 0:BL],
                        bhhn_sb[g][32 * j:32 * j + 1, :],
                        start=True, stop=False, tile_position=(32 * j, 32 * j))
                for ki, k in enumerate(K_ORDER):
                    lhsT = hT8[:, k % 2, 32 * (k // 2):32 * (k // 2) + BL]
                    for j in range(NG):
                        nc.tensor.matmul(
                            p_n[32 * j:32 * j + BL, :],
                            lhsT,
                            whh8_sb[g][:, 3072 * k + GW * j + 256:3072 * k + GW * j + 512],
                            start=False, stop=(ki == KT - 1),
                            tile_position=(0, 32 * j))
                # z'-gate: xw inject (mask-folded) + 4 DoubleRow matmuls
                for j in range(NG):
                    nc.tensor.matmul(
                        p_z[32 * j:32 * j + BL, :],
                        ident_sb[32 * j:32 * j + BL, 32 * j:32 * j + BL],
                        xw_t[32 * j:32 * j + BL, 512:768],
                        start=True, stop=False, tile_position=(32 * j, 32 * j))
                for ki, k in enumerate(K_ORDER):
                    lhsT = hT8[:, k % 2, 32 * (k // 2):32 * (k // 2) + BL]
                    for j in range(NG):
                        nc.tensor.matmul(
                            p_z[32 * j:32 * j + BL, :],
                            lhsT,
                            whh8_sb[g][:, 3072 * k + GW * j + 512:3072 * k + GW * (j + 1)],
                            start=False, stop=(ki == KT - 1),
                            tile_position=(0, 32 * j))

                # gates (strip view [128, 256]; only partitions 32j+b<8 valid)
                r_t = wpool.tile([128, 256], f16, tag="r_t")
                nc.scalar.activation(r_t[:], p_r[:], AF.Sigmoid, scale=inv_x)
                z_t = wpool.tile([128, 256], f16, tag="z_t")
                nc.scalar.activation(z_t[:], p_z[:], AF.Sigmoid, scale=inv_x)

                # n = tanh((xn + r * (hn + bhn)) / 1024)
                tn2 = wpool.tile([128, 256], f16, tag="tn2")
                nc.vector.tensor_mul(tn2[:], p_n[:], r_t[:])
                sn = wpool.tile([128, 256], f16, tag="sn")
                nc.vector.tensor_add(sn[:], tn2[:], xw_t[:, 256:512])
                n_t = wpool.tile([128, 256], f16, tag="n_t")
                nc.scalar.activation(n_t[:], sn[:], AF.Tanh, scale=inv_x)

                # h' = h + z'*(n - h)   (mask folded into z')
                d_t = wpool.tile([128, 256], f16, tag="d_t")
                nc.vector.tensor_sub(d_t[:], n_t[:], h_str[:])
                e_t = wpool.tile([128, 256], f16, tag="e_t")
                nc.vector.tensor_mul(e_t[:], d_t[:], z_t[:])

                # critical path: transpose the DELTA and accumulate into hT
                # (hT' = hT + e^T); the strip-space update h' = e + h runs in
                # the DVE idle slot while the PE transposes. fp16 rounding of
                # (h + e) is elementwise, so both copies stay bit-identical.
                tp = psumr.tile([128, 2, 128], f16, tag="tp")
                for c in range(2):
                    nc.tensor.transpose(tp[:, c, :],
                                        e_t[:, 128 * c:128 * (c + 1)],
                                        ident_sb[:])
                h_new = wpool.tile([128, 256], f16, tag="h_str", name="h_new")
                nc.vector.tensor_add(h_new[:], e_t[:], h_str[:])
                hT8_new = wpool.tile([128, 2, 128], f16, tag="hT8", name="hT8_new")
                for c in range(2):
                    nc.vector.tensor_add(hT8_new[:, c, :], tp[:, c, :], hT8[:, c, :])

                h_str, hT8 = h_new, hT8_new

            # ---- head ----------------------------------------------------
            ph = psum.tile([128, 512], f32, tag="p1", name="ph")
            for k in range(KT):
                nc.tensor.matmul(
                    ph[0:BL, 0:64],
                    hT8[:, k % 2, 32 * (k // 2):32 * (k // 2) + BL],
                    p1T_sb[:, 64 * k:64 * (k + 1)],
                    start=(k == 0), stop=(k == KT - 1),
                )
            t1s = wpool.tile([128, 64], f16, tag="t1s")
            nc.vector.tensor_add(t1s[0:BL, :], ph[0:BL, 0:64], p1b_sb[0:BL, :])
            t1 = wpool.tile([128, 64], f16, tag="t1")
            nc.scalar.activation(t1[0:BL, :], t1s[0:BL, :], AF.Tanh)

            pt1 = psumr.tile([128, 256], f16, tag="tp", name="pt1")
            nc.tensor.transpose(pt1[0:64, 0:BL], t1[0:BL, 0:64], ident_sb[0:BL, 0:BL])
            t1T = wpool.tile([64, BL], f16, tag="t1T")
            nc.vector.tensor_copy(t1T[:], pt1[0:64, 0:BL])

            pl = psum.tile([128, 512], f32, tag="p1", name="pl")
            nc.tensor.matmul(pl[0:BL, 0:2], t1T[:], p2T_sb[:], start=True, stop=True)
            lg = wpool.tile([128, 2], f32, tag="lg")
            nc.vector.tensor_add(lg[0:BL, :], pl[0:BL, 0:2], p2b_sb[0:BL, :])
            nc.sync.dma_start(d_logits[:], lg[0:BL, :])

            if debug:
                nc.sync.dma_start(d_dbg_h[:], h_str[:])
                for g in ("src", "tgt"):
                    dbg = evpool.tile([128, 3072], f16, tag="dbgxw")
                    for mi in range(n_mstrip):
                        m0 = 128 * mi
                        msz = min(128, TB - m0)
                        nc.sync.dma_start(dbg[0:msz, :], xw_dram[g][m0:m0 + msz, :])
                        nc.sync.dma_start(d_dbg_xw[g][m0:m0 + msz, :], dbg[0:msz, :])

    nc.compile()
    return nc


# ----------------------------------------------------------------------------
# entry point
# ----------------------------------------------------------------------------

@functools.lru_cache(maxsize=2)
def _cached_program(n_steps, debug):
    return build_program(n_steps, debug)


def run(inputs, n_steps=T, debug=False, trace=False):
    inputs = {k: np.asarray(v) for k, v in inputs.items()}
    nc = _cached_program(n_steps, debug)
    shared = _prep_shared(inputs, n_steps)
    emb16 = np.asarray(inputs["emb"]).astype(np.float16)
    in_maps = []
    for c in range(NCORES):
        m = dict(shared)
        m.update(_prep_core(inputs, emb16, c, n_steps))
        in_maps.append(m)
    res = run_bass_kernel_spmd(nc, in_maps, list(range(NCORES)), trace=trace)
    logits = np.concatenate([res.results[c]["logits"] for c in range(NCORES)], axis=0)
    return logits, res


def kernel(**inputs) -> np.ndarray:
    logits, _ = run(inputs)
    return logits.astype(np.float32)
